# revision 15
# baseline (speedup 1.0000x reference)
"""Additive-attention pooling (nn_Meta_Module) Trainium2 kernel — v5.

Full inputs in, full output out. Pure data-parallel over 8 NeuronCores
(batch 512 -> 64/core). Per core, a Bass/Tile kernel computes
  a    = all_memory @ U.T            (PE 128x128, bf16, [k,(b,s)] layout)
  t    = tanh(a + last @ W.T)        (DVE one-shot bias drain per window
                                      [FD=800, both k-halves] + ActE
                                      big-chunk tanh; 4 windows use the
                                      ActE fused bias-tanh path instead)
  sc   = V.T @ t                     (PE col-tiled strips, 2 batches/MM)
  P    = all_memory @ MetaW.T        (PE col-tiled strips, same PSUM bank
                                      as sc -- disjoint partition rows)
  out: scores+projections shipped to host; softmax + the tiny O(B*S*4)
  e-weighted combine + bias run on host in f64/f32.

Startup: PE warmed with dummy matmuls during the DMA fill so real MMs run
at 2.4 GHz; x streamed on sync+gpsimd DMA rings so the scalar (ActE) ring
never blocks activations.
"""
import numpy as np
import ml_dtypes
from contextlib import ExitStack

import concourse.bass as bass
import concourse.tile as tile
import concourse.mybir as mybir
from concourse import bacc
from concourse.bass_utils import run_bass_kernel_spmd

BF16 = mybir.dt.bfloat16
F32 = mybir.dt.float32
AF = mybir.ActivationFunctionType
ALU = mybir.AluOpType
NBF = ml_dtypes.bfloat16

B, S, H = 512, 200, 256
N_CORES = 8
B_LOC = B // N_CORES      # 64 batches/core
NW = B_LOC // 2           # 32 windows of 2 batches (400 cols)
NCOL = B_LOC * S          # 12800 columns per core

# windows whose bias+tanh run fused on ActE straight from PSUM (30/31 at
# the end keep the tail off the DVE-drain + chunk-tanh long path)
FUSED = (3, 11, 27, 30, 31)
# DVE-drained spans -> one big ActE tanh chunk (both k-halves in a single
# instruction), emitted right after the last drain of the span
CHUNK_AT = {2: (0, 2), 7: (4, 7), 10: (8, 10), 15: (12, 15), 19: (16, 19),
            23: (20, 23), 26: (24, 26), 29: (28, 29)}
# V/P strip bursts issued after window w's 'a' matmuls; V lags 8 windows so
# the tanh chunk feeding it is always done (no PE stall on ActE)
BURSTS = {3: (("P", 0),), 7: (("P", 1),), 8: (("V", 0),),
          11: (("P", 2),), 12: (("V", 1),), 15: (("P", 3),),
          16: (("V", 2),), 19: (("P", 4),), 20: (("V", 3),),
          23: (("P", 5),), 24: (("V", 4),), 27: (("P", 6),),
          28: (("V", 5),), 30: (("P", 7), ("V", 6)), 31: (("V", 7),)}
# x DMA chunks (col offset, cols): sized so early windows unblock fast
CHUNKS = ((0, 400), (400, 800), (1200, 1600), (2800, 2400), (5200, 3200),
          (8400, 4400))


def _ap(base_ap, offset_elems, dims):
    """AP on base_ap's tensor: dims = [(stride, count), ...] free dims."""
    p = base_ap.ap[0]
    return bass.AP(tensor=base_ap.tensor, offset=base_ap.offset + offset_elems,
                   ap=[list(p)] + [list(d) for d in dims])


def build_nc(debug=False):
    nc = bacc.Bacc("TRN2", target_bir_lowering=False, debug=debug)

    allT = [nc.dram_tensor(f"allT{h}", [128, NCOL], BF16, kind="ExternalInput")
            for h in range(2)]
    CB_d = nc.dram_tensor("CB", [128, 1024], BF16, kind="ExternalInput")
    LT_d = nc.dram_tensor("LT", [128, 128], F32, kind="ExternalInput")
    SCPT_d = nc.dram_tensor("SCPT", [128, 800], F32, kind="ExternalOutput")

    with tile.TileContext(nc) as tc, ExitStack() as ctx:
        big = ctx.enter_context(tc.tile_pool(name="big", bufs=1))
        misc = ctx.enter_context(tc.tile_pool(name="misc", bufs=1))
        apool = ctx.enter_context(tc.tile_pool(name="apool", bufs=3, space="PSUM"))
        vppool = ctx.enter_context(tc.tile_pool(name="vppool", bufs=2, space="PSUM"))

        scratch = misc.tile([128, 384], BF16, tag="scratch")
        nc.gpsimd.memset(scratch[:], 0.125)

        cb = big.tile([128, 1024], BF16, tag="cb")
        lt = big.tile([128, 128], F32, tag="lt")
        x = [big.tile([128, NCOL], BF16, tag=f"x{h}", name=f"x{h}")
             for h in range(2)]

        # The scalar+gpsimd DMA rings share a ~270 GB/s aggregate (~135
        # each when both are busy); sync's ring is ~65 GB/s (SP sequencer
        # clogged by framework semaphores) and carries only the mid-kernel
        # output. Delivery must track window order: ring loads balanced,
        # chunk c of both halves in flight together, and only the UT part
        # of CB up front (it alone gates the first matmul).
        def dma_x(eng, h, c):
            off, size = CHUNKS[c]
            eng.dma_start(x[h][:, off:off + size],
                          allT[h].ap()[:, off:off + size])

        nc.scalar.dma_start(cb[:, 0:512], CB_d.ap()[:, 0:512])
        dma_x(nc.scalar, 0, 1)
        nc.scalar.dma_start(lt[:], LT_d.ap())
        nc.scalar.dma_start(cb[:, 512:1024], CB_d.ap()[:, 512:1024])
        # warm the tanh activation table before the bulk DIRECT2Ds
        dummy = misc.tile([128, 1], BF16, tag="dummy")
        nc.scalar.activation(dummy[:], scratch[:, 0:1], AF.Tanh)
        for c in (2, 3, 4, 5):
            dma_x(nc.scalar, 0, c)
        dma_x(nc.gpsimd, 0, 0)
        for c in range(6):
            dma_x(nc.gpsimd, 1, c)

        ut = cb[:, 0:512]
        vsp = cb[:, 512:768]
        mwp = cb[:, 768:1024]

        def UT(h, k):
            return ut[:, (2 * h + k) * 128:(2 * h + k + 1) * 128]

        def VSP(k, c):
            return vsp[:, (k * 4 + c) * 32:(k * 4 + c) * 32 + 32]

        def MW(h, i8):
            return mwp[:, (h * 4 + i8) * 32:(h * 4 + i8) * 32 + 32]

        arg = big.tile([128, 2 * NCOL], BF16, tag="arg")
        tts = big.tile([128, 2 * NCOL], BF16, tag="tts")
        scpt_sb = misc.tile([128, 800], F32, tag="scpt")

        # PE warmup: ~4us of dummy matmuls so HAM unthrottles to 2.4 GHz
        # before the real stream starts. Results land in the first apool
        # slot and are overwritten by window 5's start=True matmuls.
        warm = apool.tile([128, 1024], F32, tag="a", name="warm")
        for _ in range(10):
            nc.tensor.matmul(warm[:, 0:384], scratch[:, 0:128],
                             scratch[:, 0:384], start=True, stop=True)

        vp = [None, None]
        first_touch = {}

        def emit_V(win):
            g2, wl = divmod(win, 16)
            if vp[g2] is None:
                vp[g2] = vppool.tile([128, 512], F32, tag="vp", name=f"vp{g2}")
            j, c = wl % 4, wl // 4
            ft = first_touch.setdefault((g2, j), [True])
            for k in range(2):
                nc.tensor.matmul(
                    vp[g2][32 * j:32 * j + 32, 0:2 * S], VSP(k, c),
                    tts[:, k * NCOL + 400 * win:k * NCOL + 400 * (win + 1)],
                    start=(ft[0] and k == 0), stop=(k == 1),
                    tile_position=(0, 32 * j), skip_group_check=True)
            ft[0] = False

        def emit_P(pg):
            g2, pl = divmod(pg, 16)
            if vp[g2] is None:
                vp[g2] = vppool.tile([128, 512], F32, tag="vp", name=f"vp{g2}")
            j, i8 = (pl + 2) % 4, pl // 4
            ft = first_touch.setdefault((g2, j), [True])
            for h in range(2):
                nc.tensor.matmul(
                    vp[g2][32 * j:32 * j + 32, 0:2 * S], MW(h, i8),
                    x[h][:, 400 * pg:400 * (pg + 1)],
                    start=(ft[0] and h == 0), stop=(h == 1),
                    tile_position=(0, 32 * j), skip_group_check=True)
            ft[0] = False

        def ship(g2):
            # g2=0 mid-kernel on the idle sync ring; g2=1 at the tail on
            # the scalar ring (fast HWDGE, ActE is done by then)
            eng = nc.sync if g2 == 0 else nc.scalar
            nc.vector.tensor_copy(scpt_sb[:, 400 * g2:400 * (g2 + 1)],
                                  vp[g2][:, 0:2 * S])
            eng.dma_start(SCPT_d.ap()[:, 400 * g2:400 * (g2 + 1)],
                          scpt_sb[:, 400 * g2:400 * (g2 + 1)])

        for w in range(NW):
            A = apool.tile([128, 1024], F32, tag="a", name=f"a{w}")
            for k in range(2):
                for h in range(2):
                    nc.tensor.matmul(
                        A[:, 512 * k:512 * k + 400], UT(h, k),
                        x[h][:, 400 * w:400 * (w + 1)],
                        start=(h == 0), stop=(h == 1))
            if w in FUSED:          # ActE fused bias+tanh straight from PSUM
                for k in range(2):
                    for i in range(2):
                        b = 2 * w + i
                        nc.scalar.activation(
                            tts[:, k * NCOL + S * b:k * NCOL + S * (b + 1)],
                            A[:, 512 * k + S * i:512 * k + S * (i + 1)],
                            AF.Tanh, bias=lt[:, 64 * k + b:64 * k + b + 1])
            else:                   # one DVE op drains both k-halves + bias
                in0 = _ap(A[:, :], 0, [(512, 2), (200, 2), (1, 200)])
                in1 = _ap(lt[:, :], 2 * w, [(64, 2), (1, 2), (0, 200)])
                out = _ap(arg[:, :], 400 * w, [(NCOL, 2), (200, 2), (1, 200)])
                nc.vector.tensor_tensor(out, in0, in1, ALU.add)
            if w in CHUNK_AT:       # big ActE tanh over the finished run,
                w0, w1 = CHUNK_AT[w]    # both k-halves in one instruction
                cols = 400 * (w1 + 1 - w0)
                nc.scalar.activation(
                    _ap(tts[:, :], 400 * w0, [(NCOL, 2), (1, cols)]),
                    _ap(arg[:, :], 400 * w0, [(NCOL, 2), (1, cols)]),
                    AF.Tanh)
            for kind, hh in BURSTS.get(w, ()):
                for i in range(4):
                    (emit_V if kind == "V" else emit_P)(4 * hh + i)
            if w == 21:
                ship(0)
        ship(1)
    nc.compile()
    return nc


def prep_core_inputs(all_c, last_c, U, W, V, MetaW, b_loc=B_LOC):
    x = np.ascontiguousarray(all_c.transpose(2, 0, 1)).astype(NBF)  # [H, b, S]
    m = {}
    m["allT0"] = np.ascontiguousarray(x[:128].reshape(128, b_loc * S))
    m["allT1"] = np.ascontiguousarray(x[128:].reshape(128, b_loc * S))
    l = (last_c @ W.T).astype(np.float32)
    m["LT"] = np.ascontiguousarray(
        l.T.reshape(2, 128, b_loc).transpose(1, 0, 2).reshape(128, 2 * b_loc))
    ut = U.reshape(2, 128, 2, 128).transpose(3, 2, 0, 1).reshape(128, 512)
    vsp = np.zeros((128, 256), np.float32)
    for k in range(2):
        for c in range(4):
            vsp[:, (k * 4 + c) * 32 + 16 + c] = V[128 * k:128 * (k + 1), 0]
    mwp = np.zeros((128, 2, 4, 32), np.float32)
    for h in range(2):
        for i in range(4):
            mwp[:, h, i, 4 * i:4 * i + 4] = MetaW[:, 128 * h:128 * (h + 1)].T
    mwp = mwp.reshape(128, 256)
    m["CB"] = np.ascontiguousarray(
        np.concatenate([ut, vsp, mwp], axis=1)).astype(NBF)
    return m


def postprocess_core(scpt, Metab, b_loc=B_LOC):
    """scpt [128, 800] f32: per g2-half, V rows hold scores, P rows hold the
    MetaW projections. Softmax + e-weighted combine on host."""
    sc = np.empty((b_loc, S), np.float32)
    P = np.empty((b_loc, 4, S), np.float32)
    for g2 in range(2):
        blk = scpt[:, 400 * g2:400 * (g2 + 1)]
        for wl in range(16):
            vrow = 32 * (wl % 4) + 16 + wl // 4
            prow = 32 * ((wl + 2) % 4) + 4 * (wl // 4)
            for par in range(2):
                b = 32 * g2 + 2 * wl + par
                sc[b] = blk[vrow, 200 * par:200 * (par + 1)]
                P[b] = blk[prow:prow + 4, 200 * par:200 * (par + 1)]
    sc64 = sc.astype(np.float64)
    e = np.exp(sc64 - sc64.max(axis=1, keepdims=True))
    alpha = (e / e.sum(axis=1, keepdims=True)).astype(np.float32)
    return np.einsum('bs,bms->bm', alpha, P) + Metab.reshape(1, 4)


_cache = {}


def _get_nc():
    if "nc" not in _cache:
        _cache["nc"] = build_nc()
    return _cache["nc"]


def kernel(all_memory, last_memory, U, W, V, MetaW, Metab):
    all_memory = np.asarray(all_memory, dtype=np.float32)
    last_memory = np.asarray(last_memory, dtype=np.float32)
    U = np.asarray(U, dtype=np.float32)
    W = np.asarray(W, dtype=np.float32)
    V = np.asarray(V, dtype=np.float32)
    MetaW = np.asarray(MetaW, dtype=np.float32)
    Metab = np.asarray(Metab, dtype=np.float32)
    nc = _get_nc()
    in_maps = []
    for c in range(N_CORES):
        sl = slice(c * B_LOC, (c + 1) * B_LOC)
        in_maps.append(prep_core_inputs(
            all_memory[sl], last_memory[sl], U, W, V, MetaW))
    res = run_bass_kernel_spmd(nc, in_maps, core_ids=list(range(N_CORES)))
    outs = [postprocess_core(res.results[c]["SCPT"], Metab)
            for c in range(N_CORES)]
    return np.concatenate(outs, axis=0).astype(np.float32)


# revision 17
# speedup vs baseline: 1.0834x; 1.0834x over previous
"""Additive-attention pooling (nn_Meta_Module) Trainium2 kernel — v5.

Full inputs in, full output out. Pure data-parallel over 8 NeuronCores
(batch 512 -> 64/core). Per core, a Bass/Tile kernel computes
  a    = all_memory @ U.T            (PE 128x128, bf16, [k,(b,s)] layout)
  t    = tanh(a + last @ W.T)        (DVE one-shot bias drain per window
                                      [FD=800, both k-halves] + ActE
                                      big-chunk tanh; 4 windows use the
                                      ActE fused bias-tanh path instead)
  sc   = V.T @ t                     (PE col-tiled strips, 2 batches/MM)
  P    = all_memory @ MetaW.T        (PE col-tiled strips, same PSUM bank
                                      as sc -- disjoint partition rows)
  out: scores+projections shipped to host; softmax + the tiny O(B*S*4)
  e-weighted combine + bias run on host in f64/f32.

Startup: PE warmed with dummy matmuls during the DMA fill so real MMs run
at 2.4 GHz; x streamed on sync+gpsimd DMA rings so the scalar (ActE) ring
never blocks activations.
"""
import numpy as np
import ml_dtypes
from contextlib import ExitStack

import concourse.bass as bass
import concourse.tile as tile
import concourse.mybir as mybir
from concourse import bacc
from concourse.bass_utils import run_bass_kernel_spmd

BF16 = mybir.dt.bfloat16
F32 = mybir.dt.float32
AF = mybir.ActivationFunctionType
ALU = mybir.AluOpType
NBF = ml_dtypes.bfloat16

B, S, H = 512, 200, 256
N_CORES = 8
B_LOC = B // N_CORES      # 64 batches/core
NW = B_LOC // 2           # 32 windows of 2 batches (400 cols)
NCOL = B_LOC * S          # 12800 columns per core

# windows whose bias+tanh run fused on ActE straight from PSUM (30/31 at
# the end keep the tail off the DVE-drain + chunk-tanh long path)
FUSED = (3, 11, 30, 31)
# DVE-drained spans -> one ActE tanh chunk (both k-halves in a single
# instruction), emitted right after the last drain of the span; 2-window
# granularity keeps per-chunk latency low so V bursts never stall long
CHUNK_AT = {1: (0, 1), 2: (2, 2), 5: (4, 5), 7: (6, 7), 9: (8, 9),
            10: (10, 10), 13: (12, 13), 15: (14, 15), 17: (16, 17),
            19: (18, 19), 21: (20, 21), 23: (22, 23), 25: (24, 25),
            27: (26, 27), 29: (28, 29)}
# V/P strip bursts issued after window w's 'a' matmuls; V lags 8 windows so
# the tanh chunk feeding it is always done (no PE stall on ActE)
BURSTS = {3: (("P", 0),), 7: (("P", 1),), 8: (("V", 0),),
          11: (("P", 2),), 12: (("V", 1),), 15: (("P", 3),),
          16: (("V", 2),), 19: (("P", 4),), 20: (("V", 3),),
          23: (("P", 5),), 24: (("V", 4),), 27: (("P", 6),),
          28: (("V", 5),), 30: (("P", 7), ("V", 6)), 31: (("V", 7),)}
# x DMA chunks (col offset, cols): sized so early windows unblock fast
CHUNKS = ((0, 400), (400, 800), (1200, 1600), (2800, 2400), (5200, 3200),
          (8400, 4400))


def _ap(base_ap, offset_elems, dims):
    """AP on base_ap's tensor: dims = [(stride, count), ...] free dims."""
    p = base_ap.ap[0]
    return bass.AP(tensor=base_ap.tensor, offset=base_ap.offset + offset_elems,
                   ap=[list(p)] + [list(d) for d in dims])


def build_nc(debug=False):
    nc = bacc.Bacc("TRN2", target_bir_lowering=False, debug=debug)

    allT = [nc.dram_tensor(f"allT{h}", [128, NCOL], BF16, kind="ExternalInput")
            for h in range(2)]
    CB_d = nc.dram_tensor("CB", [128, 1024], BF16, kind="ExternalInput")
    LT_d = nc.dram_tensor("LT", [128, 128], F32, kind="ExternalInput")
    SCPT_d = nc.dram_tensor("SCPT", [128, 800], F32, kind="ExternalOutput")

    with tile.TileContext(nc) as tc, ExitStack() as ctx:
        big = ctx.enter_context(tc.tile_pool(name="big", bufs=1))
        misc = ctx.enter_context(tc.tile_pool(name="misc", bufs=1))
        apool = ctx.enter_context(tc.tile_pool(name="apool", bufs=3, space="PSUM"))
        vppool = ctx.enter_context(tc.tile_pool(name="vppool", bufs=2, space="PSUM"))

        scratch = misc.tile([128, 384], BF16, tag="scratch")
        nc.gpsimd.memset(scratch[:], 0.125)

        cb = big.tile([128, 1024], BF16, tag="cb")
        lt = big.tile([128, 128], F32, tag="lt")
        x = [big.tile([128, NCOL], BF16, tag=f"x{h}", name=f"x{h}")
             for h in range(2)]

        # The scalar+gpsimd DMA rings share a ~270 GB/s aggregate (~135
        # each when both are busy); sync's ring is ~65 GB/s (SP sequencer
        # clogged by framework semaphores) and carries only the mid-kernel
        # output. Delivery must track window order: ring loads balanced,
        # chunk c of both halves in flight together, and only the UT part
        # of CB up front (it alone gates the first matmul).
        def dma_x(eng, h, c):
            off, size = CHUNKS[c]
            eng.dma_start(x[h][:, off:off + size],
                          allT[h].ap()[:, off:off + size])

        nc.scalar.dma_start(cb[:, 0:512], CB_d.ap()[:, 0:512])
        dma_x(nc.scalar, 0, 1)
        nc.scalar.dma_start(lt[:], LT_d.ap())
        nc.scalar.dma_start(cb[:, 512:1024], CB_d.ap()[:, 512:1024])
        # warm the tanh activation table before the bulk DIRECT2Ds
        dummy = misc.tile([128, 1], BF16, tag="dummy")
        nc.scalar.activation(dummy[:], scratch[:, 0:1], AF.Tanh)
        for c in (2, 3, 4, 5):
            dma_x(nc.scalar, 0, c)
        dma_x(nc.gpsimd, 0, 0)
        for c in range(6):
            dma_x(nc.gpsimd, 1, c)

        ut = cb[:, 0:512]
        vsp = cb[:, 512:768]
        mwp = cb[:, 768:1024]

        def UT(h, k):
            return ut[:, (2 * h + k) * 128:(2 * h + k + 1) * 128]

        def VSP(k, c):
            return vsp[:, (k * 4 + c) * 32:(k * 4 + c) * 32 + 32]

        def MW(h, i8):
            return mwp[:, (h * 4 + i8) * 32:(h * 4 + i8) * 32 + 32]

        arg = big.tile([128, 2 * NCOL], BF16, tag="arg")
        tts = big.tile([128, 2 * NCOL], BF16, tag="tts")
        scpt_sb = misc.tile([128, 800], F32, tag="scpt")

        # PE warmup: ~4us of dummy matmuls so HAM unthrottles to 2.4 GHz
        # before the real stream starts. Results land in the first apool
        # slot and are overwritten by window 5's start=True matmuls.
        warm = apool.tile([128, 1024], F32, tag="a", name="warm")
        for _ in range(8):
            nc.tensor.matmul(warm[:, 0:384], scratch[:, 0:128],
                             scratch[:, 0:384], start=True, stop=True)

        vp = [None, None]
        first_touch = {}

        def emit_V(win):
            g2, wl = divmod(win, 16)
            if vp[g2] is None:
                vp[g2] = vppool.tile([128, 512], F32, tag="vp", name=f"vp{g2}")
            j, c = wl % 4, wl // 4
            ft = first_touch.setdefault((g2, j), [True])
            for k in range(2):
                nc.tensor.matmul(
                    vp[g2][32 * j:32 * j + 32, 0:2 * S], VSP(k, c),
                    tts[:, k * NCOL + 400 * win:k * NCOL + 400 * (win + 1)],
                    start=(ft[0] and k == 0), stop=(k == 1),
                    tile_position=(0, 32 * j), skip_group_check=True)
            ft[0] = False

        def emit_P(pg):
            g2, pl = divmod(pg, 16)
            if vp[g2] is None:
                vp[g2] = vppool.tile([128, 512], F32, tag="vp", name=f"vp{g2}")
            j, i8 = (pl + 2) % 4, pl // 4
            ft = first_touch.setdefault((g2, j), [True])
            for h in range(2):
                nc.tensor.matmul(
                    vp[g2][32 * j:32 * j + 32, 0:2 * S], MW(h, i8),
                    x[h][:, 400 * pg:400 * (pg + 1)],
                    start=(ft[0] and h == 0), stop=(h == 1),
                    tile_position=(0, 32 * j), skip_group_check=True)
            ft[0] = False

        def ship(g2):
            # g2=0 mid-kernel on the idle sync ring; g2=1 at the tail on
            # the scalar ring (fast HWDGE, ActE is done by then)
            eng = nc.sync if g2 == 0 else nc.scalar
            nc.vector.tensor_copy(scpt_sb[:, 400 * g2:400 * (g2 + 1)],
                                  vp[g2][:, 0:2 * S])
            eng.dma_start(SCPT_d.ap()[:, 400 * g2:400 * (g2 + 1)],
                          scpt_sb[:, 400 * g2:400 * (g2 + 1)])

        for w in range(NW):
            A = apool.tile([128, 1024], F32, tag="a", name=f"a{w}")
            for k in range(2):
                for h in range(2):
                    nc.tensor.matmul(
                        A[:, 512 * k:512 * k + 400], UT(h, k),
                        x[h][:, 400 * w:400 * (w + 1)],
                        start=(h == 0), stop=(h == 1))
            if w in FUSED:          # ActE fused bias+tanh straight from PSUM
                for k in range(2):
                    for i in range(2):
                        b = 2 * w + i
                        nc.scalar.activation(
                            tts[:, k * NCOL + S * b:k * NCOL + S * (b + 1)],
                            A[:, 512 * k + S * i:512 * k + S * (i + 1)],
                            AF.Tanh, bias=lt[:, 64 * k + b:64 * k + b + 1])
            else:                   # one DVE op drains both k-halves + bias
                in0 = _ap(A[:, :], 0, [(512, 2), (200, 2), (1, 200)])
                in1 = _ap(lt[:, :], 2 * w, [(64, 2), (1, 2), (0, 200)])
                out = _ap(arg[:, :], 400 * w, [(NCOL, 2), (200, 2), (1, 200)])
                nc.vector.tensor_tensor(out, in0, in1, ALU.add)
            if w in CHUNK_AT:       # big ActE tanh over the finished run,
                w0, w1 = CHUNK_AT[w]    # both k-halves in one instruction
                cols = 400 * (w1 + 1 - w0)
                nc.scalar.activation(
                    _ap(tts[:, :], 400 * w0, [(NCOL, 2), (1, cols)]),
                    _ap(arg[:, :], 400 * w0, [(NCOL, 2), (1, cols)]),
                    AF.Tanh)
            for kind, hh in BURSTS.get(w, ()):
                for i in range(4):
                    (emit_V if kind == "V" else emit_P)(4 * hh + i)
            if w == 21:
                ship(0)
        ship(1)
    nc.compile()
    return nc


def prep_core_inputs(all_c, last_c, U, W, V, MetaW, b_loc=B_LOC):
    x = np.ascontiguousarray(all_c.transpose(2, 0, 1)).astype(NBF)  # [H, b, S]
    m = {}
    m["allT0"] = np.ascontiguousarray(x[:128].reshape(128, b_loc * S))
    m["allT1"] = np.ascontiguousarray(x[128:].reshape(128, b_loc * S))
    l = (last_c @ W.T).astype(np.float32)
    m["LT"] = np.ascontiguousarray(
        l.T.reshape(2, 128, b_loc).transpose(1, 0, 2).reshape(128, 2 * b_loc))
    ut = U.reshape(2, 128, 2, 128).transpose(3, 2, 0, 1).reshape(128, 512)
    vsp = np.zeros((128, 256), np.float32)
    for k in range(2):
        for c in range(4):
            vsp[:, (k * 4 + c) * 32 + 16 + c] = V[128 * k:128 * (k + 1), 0]
    mwp = np.zeros((128, 2, 4, 32), np.float32)
    for h in range(2):
        for i in range(4):
            mwp[:, h, i, 4 * i:4 * i + 4] = MetaW[:, 128 * h:128 * (h + 1)].T
    mwp = mwp.reshape(128, 256)
    m["CB"] = np.ascontiguousarray(
        np.concatenate([ut, vsp, mwp], axis=1)).astype(NBF)
    return m


def postprocess_core(scpt, Metab, b_loc=B_LOC):
    """scpt [128, 800] f32: per g2-half, V rows hold scores, P rows hold the
    MetaW projections. Softmax + e-weighted combine on host."""
    sc = np.empty((b_loc, S), np.float32)
    P = np.empty((b_loc, 4, S), np.float32)
    for g2 in range(2):
        blk = scpt[:, 400 * g2:400 * (g2 + 1)]
        for wl in range(16):
            vrow = 32 * (wl % 4) + 16 + wl // 4
            prow = 32 * ((wl + 2) % 4) + 4 * (wl // 4)
            for par in range(2):
                b = 32 * g2 + 2 * wl + par
                sc[b] = blk[vrow, 200 * par:200 * (par + 1)]
                P[b] = blk[prow:prow + 4, 200 * par:200 * (par + 1)]
    sc64 = sc.astype(np.float64)
    e = np.exp(sc64 - sc64.max(axis=1, keepdims=True))
    alpha = (e / e.sum(axis=1, keepdims=True)).astype(np.float32)
    return np.einsum('bs,bms->bm', alpha, P) + Metab.reshape(1, 4)


_cache = {}


def _get_nc():
    if "nc" not in _cache:
        _cache["nc"] = build_nc()
    return _cache["nc"]


def kernel(all_memory, last_memory, U, W, V, MetaW, Metab):
    all_memory = np.asarray(all_memory, dtype=np.float32)
    last_memory = np.asarray(last_memory, dtype=np.float32)
    U = np.asarray(U, dtype=np.float32)
    W = np.asarray(W, dtype=np.float32)
    V = np.asarray(V, dtype=np.float32)
    MetaW = np.asarray(MetaW, dtype=np.float32)
    Metab = np.asarray(Metab, dtype=np.float32)
    nc = _get_nc()
    in_maps = []
    for c in range(N_CORES):
        sl = slice(c * B_LOC, (c + 1) * B_LOC)
        in_maps.append(prep_core_inputs(
            all_memory[sl], last_memory[sl], U, W, V, MetaW))
    res = run_bass_kernel_spmd(nc, in_maps, core_ids=list(range(N_CORES)))
    outs = [postprocess_core(res.results[c]["SCPT"], Metab)
            for c in range(N_CORES)]
    return np.concatenate(outs, axis=0).astype(np.float32)
